# revision 30
# baseline (speedup 1.0000x reference)
"""Multi-head attention (quirky Dense(d_k) variant) on 8 trn2 NeuronCores — v3.

Sharding: batch (2) x query-blocks (4). Core c handles batch c//4 and
queries [512*(c%4), 512*(c%4+1)) for ALL 8 heads. Each core gets its
batch's full keys/values but only its query slice; outputs are disjoint
[512, 1024] blocks of y (no host-side reduction, no partial sums).

All DMA'd tensors are bf16 (inputs/weights/y) or small f32; internal
compute is fp32r with fp32 PSUM accumulation.

Heads are packed in quads (A = heads 0-3, B = 4-7) at 32-partition
offsets with zero padding:
  - score matmuls are K=32 row-groups (tile_position=(32j,0)) -> 4 heads
    concurrent in the PE array, one [128, 2048] PSUM score set per quad;
  - exp runs as two 2-bank ACT calls per (quad, Lk-tile) so the next
    quad's score matmuls overlap the second call;
  - AV matmuls are M=17 col-groups ([1|v_h] lhsT -> rowsum at 32j, head
    dims at 32j+1..16, tile_position=(0,32j)), accumulated in PSUM over
    the 16 Lk tiles; AV emission lags one stage so the PE FIFO never
    parks an exp-gated AV ahead of score matmuls;
  - v/k projections for later tiles are sprinkled INTO the attention
    loop as small units so the PE FIFO never head-blocks on input DMAs
    still in flight;
  - normalization is one strided reciprocal (rowsum rows at partition
    stride 32) + a K=4 expander matmul + one multiply per quad;
  - output projection runs straight off the packed layout: zero-padded
    Wo rows null the junk partitions.

PSUM: scores 4 banks + AV 2 + projection/bcast/y scratch 2 = 8.
"""

import math
import os
import sys

sys.path.insert(0, "/opt/trn_rl_repo")

import numpy as np
import ml_dtypes

import concourse.bass as bass
import concourse.mybir as mybir
import concourse.tile as tile
from concourse import bacc
from concourse.bass_utils import run_bass_kernel_spmd

H = 8
DM = 1024          # d_model
DK = 128           # projection width (d_model / h)
HD = 16            # per-head dim
B, L = 2, 2048
LQ = 512           # per-core query slice
NT = 16            # Lk tiles of 128
SCALE = 1.0 / math.sqrt(float(DK))   # reference scales by sqrt(d_k)=sqrt(128)
F32 = mybir.dt.float32
F32R = mybir.dt.float32r
BF16 = mybir.dt.bfloat16

# brow layout (single-partition f32r row): bv | bo | ones
BV_O, BO_O, ONES_O = 0, 128, 1152
BROW_N = 1664

_CACHE = {}


def _build_nc():
    nc = bacc.Bacc(None, target_bir_lowering=False)

    xq = nc.declare_dram_parameter("xq", [DM, LQ], BF16, isOutput=False)
    xk = nc.declare_dram_parameter("xk", [DM, L], BF16, isOutput=False)
    xv = nc.declare_dram_parameter("xv", [DM, L], BF16, isOutput=False)
    # packed [p, (g c d64)]: g in {qA,qB,kA,kB}, value = W_g[c*128+p, d]
    # (head j's 16 cols contiguous; expanded on device to 32-aligned slots)
    wqk = nc.declare_dram_parameter("wqk", [128, 4 * 8 * 64], BF16, isOutput=False)
    # [p, (c d)] = Wv[c*128+p, d]
    wv = nc.declare_dram_parameter("wv", [128, 8 * 128], BF16, isOutput=False)
    # packed [p64, (q n)] = Wo[16*(4q+j)+d, n] at p64=16j+d; expanded on
    # device to the zero-padded 32-aligned row layout
    woz = nc.declare_dram_parameter("woz", [64, 2 * DM], BF16, isOutput=False)
    # per-partition bias columns: bqzA, bqzB, bkzA, bkzB (quad-padded)
    bcols = nc.declare_dram_parameter("bcols", [128, 4], F32, isOutput=False)
    brow = nc.declare_dram_parameter("brow", [1, BROW_N], F32R, isOutput=False)
    # expander: esel[j, 32j:32j+32] = 1 -> K=4 matmul broadcasts row j of the
    # rhs to partitions 32j..32j+31
    esel = nc.declare_dram_parameter("esel", [4, 128], F32R, isOutput=False)
    # gather: gsel[32j, j] = 1 -> K=128 matmul pulls rowsum rows 0/32/64/96
    # into adjacent partitions 0..3
    gsel = nc.declare_dram_parameter("gsel", [128, 4], F32R, isOutput=False)
    y = nc.declare_dram_parameter("y", [LQ, DM], BF16, isOutput=True)

    dbg = os.environ.get("KERNEL_DEBUG", "0") == "1"
    if dbg:
        ktz_d = nc.declare_dram_parameter("ktz_d", [2 * 128, L], BF16, isOutput=True)
        qtz_d = nc.declare_dram_parameter("qtz_d", [2 * 128, LQ], BF16, isOutput=True)
        vb_d = nc.declare_dram_parameter("vb_d", [128, NT * 8 * 17], BF16, isOutput=True)
        av_d = nc.declare_dram_parameter("av_d", [2 * 128, LQ], F32R, isOutput=True)
        otn_d = nc.declare_dram_parameter("otn_d", [2 * 128, LQ], BF16, isOutput=True)

    Exp = mybir.ActivationFunctionType.Exp

    with tile.TileContext(nc) as tc:
        with (
            tc.tile_pool(name="const", bufs=1) as constp,
            tc.tile_pool(name="qk", bufs=1) as qkpool,
            tc.tile_pool(name="pp", bufs=2, space="PSUM") as ppool,
            tc.tile_pool(name="sc", bufs=2, space="PSUM") as spool,
            tc.tile_pool(name="ep", bufs=4) as epool,
            tc.tile_pool(name="yp", bufs=2) as ypool,
            tc.tile_pool(name="mp", bufs=2) as mpool,
        ):
            # ---- constants + inputs, ONE sync queue in strict first-use
            # order; per-DMA fixed cost dominates small transfers so each is
            # as large as dependency granularity allows (column quarters for
            # k/v so early attention tiles unblock after 1/4 of the bytes) ----
            wqk_sb = constp.tile([128, 4, 8, 128], BF16)
            wqk_pk = constp.tile([128, 4, 8, 64], BF16)
            wv_sb = constp.tile([128, 8, 128], BF16)
            woz_sb = constp.tile([128, 2, DM], BF16)
            bcols_sb = constp.tile([128, 4], F32)
            brow_sb = constp.tile([1, BROW_N], F32R)
            esel_sb = constp.tile([4, 128], F32R)
            gsel_sb = constp.tile([128, 4], F32R)
            xq_sb = constp.tile([128, 8, LQ], BF16)
            xk_sb = constp.tile([128, 8, L], BF16)
            xv_sb = constp.tile([128, 8, L], BF16)

            def ones(n):
                return brow_sb[0:1, ONES_O : ONES_O + n]

            # PE warmup: zero matmuls while input DMAs stream keep the HAM
            # clock gate open so the first projections run at full rate; the
            # wmup memset is the FIRST DVE op so warmup starts immediately
            wmup = mpool.tile([128, 512], BF16, tag="wm", name="wmup")
            nc.vector.memset(wmup, 0.0)
            wps = ppool.tile([128, 512], F32, tag="pp", name="wps")
            for _ in range(14):
                nc.tensor.matmul(
                    wps, lhsT=wmup[:, 0:128], rhs=wmup, start=True, stop=True
                )

            nc.sync.dma_start(
                out=wqk_pk, in_=wqk[:].rearrange("p (g c d) -> p g c d", g=4, c=8)
            )
            nc.sync.dma_start(out=bcols_sb, in_=bcols[:])
            # expand packed weights into 32-aligned head slots (zeros between)
            nc.vector.memset(wqk_sb.rearrange("p g c (j d) -> p g c j d", j=4)[:, :, :, :, 16:32], 0.0)
            with nc.allow_low_precision(reason="bf16 weight expand"):
                nc.vector.tensor_copy(
                    wqk_sb.rearrange("p g c (j d) -> p g c j d", j=4)[:, :, :, :, 0:16],
                    wqk_pk.rearrange("p g c (j d) -> p g c j d", j=4),
                )

            def k_quarter(nb):
                nc.sync.dma_start(
                    out=xk_sb[:, :, nb * 512 : (nb + 1) * 512],
                    in_=xk[:, nb * 512 : (nb + 1) * 512].rearrange(
                        "(c p) n -> p c n", p=128
                    ),
                )

            def v_quarter(g):
                nc.sync.dma_start(
                    out=xv_sb[:, :, g * 512 : (g + 1) * 512],
                    in_=xv[:, g * 512 : (g + 1) * 512].rearrange(
                        "(c p) n -> p c n", p=128
                    ),
                )

            k_quarter(0)
            nc.sync.dma_start(out=xq_sb, in_=xq[:].rearrange("(c p) n -> p c n", p=128))
            nc.sync.dma_start(out=brow_sb, in_=brow[:])
            nc.sync.dma_start(out=wv_sb, in_=wv[:].rearrange("p (c d) -> p c d", c=8))
            v_quarter(0)
            for i in range(1, 4):
                k_quarter(i)
                v_quarter(i)
            nc.vector.memset(woz_sb, 0.0)
            for j in range(4):
                nc.sync.dma_start(
                    out=woz_sb[32 * j + 1 : 32 * j + 17],
                    in_=woz[16 * j : 16 * j + 16].rearrange("p (q n) -> p q n", q=2),
                )
            nc.sync.dma_start(out=esel_sb, in_=esel[:])
            nc.sync.dma_start(out=gsel_sb, in_=gsel[:])

            kTz = [qkpool.tile([128, L], BF16, name=f"ktz{q}") for q in range(2)]
            qTz = [qkpool.tile([128, LQ], BF16, name=f"qtz{q}") for q in range(2)]
            # [p(Lk within tile), t, h, 17]: col 0 = ones (rowsum), 1..16 = v dims
            vball = qkpool.tile([128, NT, 8, 17], BF16, name="vball")
            nc.vector.memset(vball[:, :, :, 0:1], 1.0)

            def q_proj(quad):
                ps = ppool.tile([128, LQ], F32, tag="pp", name="psq")
                for cc in range(8):
                    nc.tensor.matmul(
                        ps,
                        lhsT=wqk_sb[:, quad, cc, :],
                        rhs=xq_sb[:, cc, :],
                        start=(cc == 0),
                        stop=(cc == 7),
                    )
                with nc.allow_low_precision(reason="bf16 qTz"):
                    nc.vector.tensor_scalar_add(
                        qTz[quad], ps, bcols_sb[:, quad : quad + 1]
                    )

            kstate = {}

            def k_half(quad, nb, half):
                if half == 0:
                    kstate[(quad, nb)] = ppool.tile(
                        [128, 512], F32, tag="pp", name="psk"
                    )
                ps = kstate[(quad, nb)]
                for cc in range(4 * half, 4 * half + 4):
                    nc.tensor.matmul(
                        ps,
                        lhsT=wqk_sb[:, 2 + quad, cc, :],
                        rhs=xk_sb[:, cc, nb * 512 : (nb + 1) * 512],
                        start=(cc == 0),
                        stop=(cc == 7),
                    )
                if half == 1:
                    with nc.allow_low_precision(reason="bf16 kTz"):
                        nc.vector.tensor_scalar_add(
                            kTz[quad][:, nb * 512 : (nb + 1) * 512],
                            ps,
                            bcols_sb[:, 2 + quad : 3 + quad],
                        )

            def k_proj(quad, nb):
                k_half(quad, nb, 0)
                k_half(quad, nb, 1)

            # v projection as 16 per-Lk-tile units emitted inside the
            # attention loop (natural [Lk, dim] layout, bias via K=1 matmul,
            # per-tile evac into the [16|ones] packed vball layout)
            vstate = {"psv": None}
            bvt_sb = constp.tile([128, 128], F32)

            def bvt_make():
                ps = ppool.tile([128, 512], F32, tag="pp", name="bvt")
                nc.tensor.matmul(
                    ps[:, 0:128], lhsT=ones(128),
                    rhs=brow_sb[0:1, BV_O : BV_O + 128], start=True, stop=True,
                )
                nc.vector.tensor_copy(bvt_sb, ps[:, 0:128])

            def v_unit(t):
                lk = t % 4
                if lk == 0:
                    vstate["psv"] = ppool.tile([128, 512], F32, tag="pp", name="psv")
                psv = vstate["psv"]
                o = psv[:, lk * 128 : (lk + 1) * 128]
                for cc in range(8):
                    nc.tensor.matmul(
                        o,
                        lhsT=xv_sb[:, cc, t * 128 : (t + 1) * 128],
                        rhs=wv_sb[:, cc, :],
                        start=(cc == 0),
                        stop=(cc == 7),
                    )
                with nc.allow_low_precision(reason="bf16 vball"):
                    nc.vector.tensor_add(
                        vball[:, t, :, 1:17],
                        o.rearrange("p (h d) -> p h d", h=8),
                        bvt_sb.rearrange("p (h d) -> p h d", h=8),
                    )

            # ---- prologue projections: ONLY quad A's q/k-nb0 precede the
            # first score matmuls in the PE FIFO; everything else is filler
            # consumed one unit per attention slot ----
            k_proj(0, 0)
            q_proj(0)

            # fillers are consumed at the END of each slot (after the exp
            # emission) so nothing but quad A's own work precedes the first
            # exp in the PE FIFO
            fillers = {0: lambda: (k_proj(1, 0), q_proj(1), bvt_make())}
            fillers_pre = {1: lambda: v_unit(0)}
            for u in range(1, NT):
                fillers[2 * u - 1] = (lambda uu: (lambda: v_unit(uu)))(u)
            ksched = {2: (0, 1, 0), 4: (0, 1, 1), 6: (1, 1, 0), 8: (1, 1, 1),
                      10: (0, 2, 0), 12: (0, 2, 1), 14: (1, 2, 0), 16: (1, 2, 1),
                      18: (0, 3, 0), 20: (0, 3, 1), 22: (1, 3, 0), 24: (1, 3, 1)}
            for s, (quad, nb, half) in ksched.items():
                fillers[s] = (
                    lambda q_, n_, h_: (lambda: k_half(q_, n_, h_))
                )(quad, nb, half)

            # ---- attention ----
            # per-quad SBUF accumulators; AV matmuls land per-slot in the
            # just-read score bank and a single DVE add folds them in. This
            # keeps ALL 6 score banks in a bufs=3 rotation so every exp's
            # dependencies are satisfied ~2 slots early -> ACT back-to-back.
            oacc = [qkpool.tile([128, LQ], F32R, name=f"oacc{q}") for q in range(2)]

            def do_av(quad, t, et, Sav, js=(0, 1, 2, 3)):
                for j in js:
                    nc.tensor.matmul(
                        Sav[32 * j : 32 * j + 17, 0:512],
                        lhsT=vball[:, t, 4 * quad + j, :],
                        rhs=et[:, j * 512 : (j + 1) * 512],
                        start=True,
                        stop=True,
                        skip_group_check=True,
                        tile_position=(0, 32 * j),
                    )

            def av_fold(quad, Sav, first):
                with nc.allow_low_precision(reason="f32r oacc"):
                    if first:
                        nc.vector.tensor_copy(oacc[quad], Sav[:, 0:512])
                    else:
                        nc.vector.tensor_add(oacc[quad], oacc[quad], Sav[:, 0:512])

            # software pipeline: AV(quad, t) is emitted AFTER the next quad's
            # score matmuls + ACT so the PE FIFO never parks AV (gated on the
            # 2nd exp call) ahead of score matmuls (gated only on the 1st).
            # v units and later k projections are sprinkled in as PE filler
            # that executes in ACT shadows.
            def do_scores(quad, t, Sh, js):
                for j in js:
                    nc.tensor.matmul(
                        Sh[:, (j % 2) * 512 : (j % 2) * 512 + 512],
                        lhsT=kTz[quad][32 * j : 32 * j + 32, t * 128 : (t + 1) * 128],
                        rhs=qTz[quad][32 * j : 32 * j + 32, :],
                        start=True,
                        stop=True,
                        tile_position=(32 * j, 0),
                    )

            pend = None
            for t in range(NT):
                for quad in range(2):
                    s = 2 * t + quad
                    fp = fillers_pre.get(s)
                    if fp is not None:
                        fp()
                    S01 = spool.tile([128, 1024], F32, tag="sa", name="S01")
                    do_scores(quad, t, S01, (0, 1))
                    if pend is not None:
                        pq, pt, pet, pS = pend
                        do_av(pq, pt, pet, pS, js=(0, 1))
                    S23 = spool.tile([128, 1024], F32, tag="sb", name="S23", bufs=1)
                    do_scores(quad, t, S23, (2, 3))
                    if pend is not None:
                        do_av(pq, pt, pet, pS, js=(2, 3))
                        av_fold(pq, pS, first=(pt == 0))
                    et = epool.tile([128, 2048], BF16, tag="e", name="et")
                    with nc.allow_low_precision(reason="bf16 softmax weights"):
                        nc.scalar.activation(et[:, 0:1024], S01, Exp, scale=SCALE)
                        nc.scalar.activation(et[:, 1024:2048], S23, Exp, scale=SCALE)
                    f = fillers.get(s)
                    if f is not None:
                        f()
                    pend = (quad, t, et, S01)
            pq, pt, pet, pS = pend

            # ---- normalize: oT[32j+1+d] /= rowsum[32j], batched per quad ----
            oTn = [qkpool.tile([128, LQ], BF16, name=f"otn{q}") for q in range(2)]

            def normalize(quad):
                with nc.allow_low_precision(reason="f32r/bf16 normalize chain"):
                    if dbg:
                        nc.sync.dma_start(
                            out=av_d[quad * 128 : (quad + 1) * 128, :], in_=oacc[quad]
                        )
                    gps = ppool.tile([128, 512], F32, tag="pp", name="gps")
                    nc.tensor.matmul(
                        gps[0:4, :], lhsT=gsel_sb, rhs=oacc[quad], start=True, stop=True
                    )
                    rs4 = mpool.tile([4, LQ], F32R, tag="rs", name="rs4")
                    nc.vector.reciprocal(rs4, gps[0:4, :])
                    rbt = ppool.tile([128, 512], F32, tag="pp", name="rbt")
                    nc.tensor.matmul(rbt, lhsT=esel_sb, rhs=rs4, start=True, stop=True)
                    nc.vector.tensor_mul(oTn[quad], oacc[quad], rbt)

            # quad A's accumulator is complete one slot before quad B's:
            # normalize it while the final AV drains
            normalize(0)
            do_av(pq, pt, pet, pS, js=(0, 1, 2, 3))
            av_fold(pq, pS, first=False)
            normalize(1)

            if dbg:
                for q in range(2):
                    nc.sync.dma_start(out=ktz_d[q * 128 : (q + 1) * 128, :], in_=kTz[q])
                    nc.sync.dma_start(out=qtz_d[q * 128 : (q + 1) * 128, :], in_=qTz[q])
                    nc.sync.dma_start(out=otn_d[q * 128 : (q + 1) * 128, :], in_=oTn[q])
                nc.sync.dma_start(out=vb_d[:], in_=vball.rearrange("p t h s -> p (t h s)"))

            # ---- output projection: y = oT^T @ WoZ + bo, natural row-major ----
            for ch in range(4):
                ysb = ypool.tile([128, DM], BF16, tag="y", name="ysb")
                py = spool.tile([128, 1024], F32, tag="sa", name="py")
                for hf in range(2):
                    ph = py[:, hf * 512 : (hf + 1) * 512]
                    nc.tensor.matmul(
                        ph,
                        lhsT=ones(128),
                        rhs=brow_sb[0:1, BO_O + hf * 512 : BO_O + (hf + 1) * 512],
                        start=True,
                        stop=False,
                    )
                    for quad in range(2):
                        nc.tensor.matmul(
                            ph,
                            lhsT=oTn[quad][:, ch * 128 : (ch + 1) * 128],
                            rhs=woz_sb[:, quad, hf * 512 : (hf + 1) * 512],
                            start=False,
                            stop=(quad == 1),
                        )
                with nc.allow_low_precision(reason="bf16 y output"):
                    if ch % 2 == 0:
                        nc.vector.tensor_copy(ysb, py)
                    else:
                        nc.scalar.copy(ysb, py)
                nc.sync.dma_start(out=y[ch * 128 : (ch + 1) * 128, :], in_=ysb)

    nc.finalize()
    return nc


def _get_nc():
    if "nc" not in _CACHE:
        _CACHE["nc"] = _build_nc()
    return _CACHE["nc"]


def _quad_pad_w(wcols):
    # [1024, 64] -> [1024, 128]: head j's 16 cols at 32j..32j+15, rest 0
    out = np.zeros((DM, 128), np.float32)
    for j in range(4):
        out[:, 32 * j : 32 * j + 16] = wcols[:, 16 * j : 16 * j + 16]
    return out


def _quad_pad_b(bcols64):
    out = np.zeros((128,), np.float32)
    for j in range(4):
        out[32 * j : 32 * j + 16] = bcols64[16 * j : 16 * j + 16]
    return out


def _woz_half(wo_rows):
    # [64, 1024] -> [128, 1024]: head j's 16 rows at 32j+1..32j+16, rest 0
    out = np.zeros((128, DM), np.float32)
    for j in range(4):
        out[32 * j + 1 : 32 * j + 17] = wo_rows[16 * j : 16 * j + 16]
    return out


def make_in_maps(queries, keys, values, Wq, bq, Wk, bk, Wv, bv, Wo, bo, **_unused):
    bf = ml_dtypes.bfloat16

    wz = [Wq[:, 0:64], Wq[:, 64:128], Wk[:, 0:64], Wk[:, 64:128]]
    wqk_a = np.ascontiguousarray(
        np.stack(wz).reshape(4, 8, 128, 64).transpose(2, 0, 1, 3).reshape(128, 2048).astype(bf)
    )
    wv_a = np.ascontiguousarray(
        Wv.reshape(8, 128, 128).transpose(1, 0, 2).reshape(128, 1024).astype(bf)
    )
    woz_a = np.ascontiguousarray(
        np.concatenate([Wo[0:64], Wo[64:128]], axis=1).astype(bf)
    )
    bcols_a = np.ascontiguousarray(
        np.stack(
            [
                _quad_pad_b(bq[0:64]),
                _quad_pad_b(bq[64:128]),
                _quad_pad_b(bk[0:64]),
                _quad_pad_b(bk[64:128]),
            ],
            axis=1,
        )
    )
    brow_a = np.zeros((1, BROW_N), np.float32)
    brow_a[0, BV_O : BV_O + 128] = bv
    brow_a[0, BO_O : BO_O + DM] = bo
    brow_a[0, ONES_O : ONES_O + 512] = 1.0
    esel_a = np.zeros((4, 128), np.float32)
    gsel_a = np.zeros((128, 4), np.float32)
    for j in range(4):
        esel_a[j, 32 * j : 32 * j + 32] = 1.0
        gsel_a[32 * j, j] = 1.0

    xqt = [np.ascontiguousarray(queries[b].T).astype(bf) for b in range(B)]
    xkt = [np.ascontiguousarray(keys[b].T).astype(bf) for b in range(B)]
    xvt = [np.ascontiguousarray(values[b].T).astype(bf) for b in range(B)]

    in_maps = []
    for core in range(8):
        b, sl = core // 4, core % 4
        in_maps.append(
            {
                "xq": np.ascontiguousarray(xqt[b][:, sl * LQ : (sl + 1) * LQ]),
                "xk": xkt[b],
                "xv": xvt[b],
                "wqk": wqk_a,
                "wv": wv_a,
                "woz": woz_a,
                "bcols": bcols_a,
                "brow": brow_a,
                "esel": esel_a,
                "gsel": gsel_a,
            }
        )
    return in_maps


def kernel(queries, keys, values, Wq, bq, Wk, bk, Wv, bv, Wo, bo, **_unused):
    queries = np.asarray(queries, dtype=np.float32)
    keys = np.asarray(keys, dtype=np.float32)
    values = np.asarray(values, dtype=np.float32)
    Wq, Wk, Wv = (np.asarray(a, dtype=np.float32) for a in (Wq, Wk, Wv))
    Wo = np.asarray(Wo, dtype=np.float32)
    bq, bk, bv, bo = (np.asarray(a, dtype=np.float32) for a in (bq, bk, bv, bo))

    nc = _get_nc()
    in_maps = make_in_maps(queries, keys, values, Wq, bq, Wk, bk, Wv, bv, Wo, bo)
    res = run_bass_kernel_spmd(nc, in_maps, core_ids=list(range(8)))
    out = np.zeros((B, L, DM), np.float32)
    for core in range(8):
        b, sl = core // 4, core % 4
        out[b, sl * LQ : (sl + 1) * LQ, :] = res.results[core]["y"].astype(np.float32)
    return out
